# revision 1
# baseline (speedup 1.0000x reference)
"""CTC loss (reduction='mean') on 8 Trainium2 NeuronCores — v3.

Strategy (pure batch data-parallelism, 16 samples per core):

  * The memory-bound part — logZ[b,t] = log sum_c exp(pred[b,t,c]) — must
    touch every element of pred once.  HW-probed rates per core: DMA bf16
    ~500 GB/s (3.34 us per [128,6626] tile); ScalarE ACT(Exp,accum) is
    1 elem/lane/cycle regardless of dtype (6.2 us/tile); DVE tensor_scalar
    is 2-4x for 16-bit.  pred is uploaded once as bf16(pred * 128*log2e)
    and the row sums are computed on TWO engines concurrently:
      - samples 0..NA-1: ScalarE exp+accumulate with the free affine
        scale=1/(128*log2e) undoing the host pre-scale (exact exp);
      - samples NA..15: VectorE two-instruction exp2 bit trick:
          i16  = int16(y + B)          (tensor_scalar_add, y = x*128*log2e)
          sum += bitcast_bf16(i16)     (tensor_scalar mult/add + accum)
        bitcast(int16(128*(127+t))) = 2^floor(t)*(1+frac(t)), a
        mantissa-linear exp2; B is offset by -7.3642 int units making it
        unbiased over uniform frac (HW-validated: matches the host
        simulation to 4e-7, true expsum to ~1e-3/row, mean +8e-5).
    Per-element noise averages down by 1/sqrt(6625) per row; residual
    loss error ~1e-4 vs the 2e-2 gate.  The CTC DP below consumes
    full-precision f32 logits gathered on the host.

  * Sequential part: CTC forward AND backward DP (fwd rows 0-15, bwd
    rows 16-31 of the same ops) in the probability domain on
    p~ = exp(glog - rhat), glog[b,t,s] = pred[b,t,ext[b,s]].  Per state,
    one tensor_tensor_scan computes all 63 time steps in ONE instruction
    (DVE small ops are ~250-430ns dispatch-bound, so fewer/bigger wins):
      blanks (even s, never skip-reachable):
        A[t,s] = (A[t-1,s-1] + A[t-1,s])*p~[t,s]          (scan)
      labels (odd s):
        w_t    = skip[s]*A[t,s-2] + A[t,s-1]              (scalar_tensor_tensor)
        A[t,s] = (w_{t-1} + A[t-1,s])*p~[t,s]             (scan)
    76 ops total.  Host pre-scales each sample/direction by
    rhat = mean_t log sum_s p~ so renorm-free f32 stays in range; the
    exact correction 64*(rhatF+rhatB) is added back on the host.

  * Host: index prep, glog gather, DP inits, fwd/bwd junction at t=63
    (lik = sum_s alpha_63*beta_63), final combine
    loss = mean_b( (sum_t logZ[b,t] - dp_log[b]) / L_b ).
"""

from contextlib import ExitStack

import numpy as np
import ml_dtypes

import concourse.bacc as bacc
import concourse.tile as tile
from concourse import mybir
from concourse.bass_utils import run_bass_kernel_spmd

B, T, C, Lmax = 128, 128, 6625, 25
C2 = C + 1  # pad to even free dim (pad col = 0.0, subtracted on host)
S = 2 * Lmax + 1  # 51 extended-label states
NCORES = 8
BL = B // NCORES  # 16 samples per core
BL2 = 2 * BL  # fwd rows 0..15, bwd rows 16..31
TH = T // 2  # 64: junction at t=63; both directions run 63 rounds
NA = 11  # samples 0..NA-1 -> ScalarE exp; NA..15 -> DVE bit trick
CRUSH = -50.0  # logit for states beyond 2L (unreachable by the answer)

LOG2E = 1.4426950408889634
A16 = 128.0 * LOG2E
B16 = 127.0 * 128.0 - 7.364191473891893  # unbiased over uniform frac
TRICK_PAD = 0.97265625  # trick(0.0) = bitcast(int16(round(B16))), exact

_TRACE = False
_LAST_RESULTS = None
_PROGRAM_CACHE = {}

f32 = mybir.dt.float32
bf16 = mybir.dt.bfloat16
i16 = mybir.dt.int16


def _build_program(
    reps: int = 1, na: int = NA, skip_trick: bool = False,
    skip_dp: bool = False, skip_act: bool = False,
) -> bacc.Bacc:
    """reps>1 wraps the whole body in a hardware loop (timing probes).
    skip_trick/skip_dp build timing-bisection variants (wrong results)."""
    Act = mybir.ActivationFunctionType
    Alu = mybir.AluOpType
    NV = BL - na  # DVE-trick samples

    nc = bacc.Bacc("TRN2", target_bir_lowering=False, debug=False)
    pr_d = nc.dram_tensor("pred16", [BL * T, C2], bf16, kind="ExternalInput").ap()
    glog_d = nc.dram_tensor("glog", [BL2, S * TH], f32, kind="ExternalInput").ap()
    m2_d = nc.dram_tensor("m2", [BL2, S], f32, kind="ExternalInput").ap()
    a0_d = nc.dram_tensor("alpha0", [BL2, S], f32, kind="ExternalInput").ap()
    sume_d = nc.dram_tensor("sumexp", [T, BL], f32, kind="ExternalOutput").ap()
    af_d = nc.dram_tensor("alpha_f", [BL2, S], f32, kind="ExternalOutput").ap()

    with tile.TileContext(nc) as tc, ExitStack() as ctx:
        io = ctx.enter_context(tc.tile_pool(name="io", bufs=3))
        iov = ctx.enter_context(tc.tile_pool(name="iov", bufs=max(NV, 1)))
        sc = ctx.enter_context(tc.tile_pool(name="scratch", bufs=1))
        sm = ctx.enter_context(tc.tile_pool(name="small", bufs=1))

        stats = sm.tile([T, BL], f32)
        glog_t = sm.tile([BL2, S * TH], f32)
        ps = sm.tile([BL2, S * TH], f32)  # p~ series, state-major
        AT = sm.tile([BL2, (S + 2) * TH], f32)  # alpha series, 2 zero states
        m2t = sm.tile([BL2, S], f32)
        a0t = sm.tile([BL2, S], f32)
        wt = sm.tile([BL2, TH], f32)
        aft = sm.tile([BL2, S], f32)
        ex8 = sc.tile([T, C2], mybir.dt.float8e4, tag="ex8")  # dummy ACT out
        it16 = sc.tile([T, C2], i16, tag="it16")
        ot16 = sc.tile([T, C2], bf16, tag="ot16")

        def body():
            # DP inputs first so the (one) exp over glog lands early on ScalarE.
            nc.sync.dma_start(glog_t[:], glog_d[:, :])
            nc.sync.dma_start(m2t[:], m2_d[:, :])
            nc.sync.dma_start(a0t[:], a0_d[:, :])
            nc.scalar.activation(ps[:], glog_t[:], Act.Exp)

            # Stream DMAs.  ACT tiles rotate in a 3-buf pool (consumed at
            # ~6.2us/tile vs ~3.8us/tile delivery); trick tiles are all
            # resident (bufs=NV) so their DMAs never stall the FIFO ring
            # behind the ~33us DP that precedes trick consumption.  Mild
            # interleave keeps every consumer ahead of its delivery.
            order = []
            vi = iter(range(na, BL))
            for k in range(na):
                order.append(k)
                if k % 2 == 0 and k > 0:
                    order.extend([j for j in [next(vi, None)] if j is not None])
            order.extend(vi)
            tls = {}
            for k in order:
                pool, tag = (io, "pred16") if k < na else (iov, "trick16")
                tl = pool.tile([T, C2], bf16, tag=tag)
                nc.sync.dma_start(tl[:], pr_d[k * T : (k + 1) * T, :])
                tls[k] = tl

            def sample_slice(k):
                return tls[k][:]

            if skip_trick or skip_act:
                nc.vector.memset(stats[:], 0.0)
            # ScalarE: exact exp + free accumulate (affine undoes the A16).
            for k in range(0 if skip_act else na):
                nc.scalar.activation(
                    ex8[:], sample_slice(k), Act.Exp, scale=1.0 / A16,
                    accum_out=stats[:, k : k + 1],
                )

            # VectorE: CTC fwd+bwd DP via per-state scans.
            if skip_dp:
                nc.vector.tensor_copy(aft[:], a0t[:])
                run_dp = False
            else:
                run_dp = True
            nc.vector.memset(AT[:, 0 : 2 * TH], 0.0)
            nc.vector.tensor_copy(AT[:, 2 * TH :: TH], a0t[:])  # t=0 column
            for s in range(S if run_dp else 0):
                base = (s + 2) * TH
                if s % 2 == 1:  # label state: skip term exists
                    nc.vector.scalar_tensor_tensor(
                        wt[:, 0 : TH - 1],
                        AT[:, base - 2 * TH : base - 2 * TH + TH - 1],
                        m2t[:, s : s + 1],
                        AT[:, base - TH : base - TH + TH - 1],
                        Alu.mult,
                        Alu.add,
                    )
                    d0 = wt[:, 0 : TH - 1]
                else:  # blank state: w is just the s-1 series
                    d0 = AT[:, base - TH : base - TH + TH - 1]
                nc.vector.tensor_tensor_scan(
                    AT[:, base + 1 : base + TH],
                    d0,
                    ps[:, s * TH + 1 : (s + 1) * TH],
                    AT[:, base : base + 1],
                    Alu.add,
                    Alu.mult,
                )
            if run_dp:
                nc.vector.tensor_copy(aft[:], AT[:, 3 * TH - 1 :: TH])  # t=63

            # VectorE: exp2 bit-trick row sums for the remaining tiles.
            if not skip_trick:
                for k in range(NV):
                    nc.vector.tensor_scalar(
                        it16[:], sample_slice(na + k), 1.0, B16,
                        Alu.mult, Alu.add,
                    )
                    nc.vector.tensor_scalar(
                        ot16[:], it16[:].bitcast(bf16), 1.0, 0.0,
                        Alu.mult, Alu.add,
                        accum_out=stats[:, na + k : na + k + 1],
                    )

            # Output DMAs on the SWDGE ring: they depend on the last
            # compute, and on the sync HWDGE ring they would block the next
            # pass's prefetch stream (FIFO per ring).
            nc.gpsimd.dma_start(sume_d[:, :], stats[:])
            nc.gpsimd.dma_start(af_d[:, :], aft[:])

        if reps == 1:
            body()
        else:
            with tc.For_i(0, reps):
                body()
    nc.compile()
    return nc


def _get_program() -> bacc.Bacc:
    if "nc" not in _PROGRAM_CACHE:
        _PROGRAM_CACHE["nc"] = _build_program()
    return _PROGRAM_CACHE["nc"]


def _host_prep(pred, label, L):
    """Extended labels, skip premasks, prescaled fwd/bwd p-series, inits."""
    ext = np.zeros((B, S), np.int64)
    ext[:, 1::2] = label
    prev2 = np.zeros_like(ext)
    prev2[:, 2:] = ext[:, :-2]
    skip = (ext != 0) & (ext != prev2) & (np.arange(S)[None, :] >= 2)

    # Host gather of the extended-label logits; crush states beyond 2L
    # (they never reach the readout states and only pollute the row sums).
    glog = np.take_along_axis(pred, ext[:, None, :], axis=2).astype(np.float32)
    smask = np.arange(S)[None, :] > (2 * L)[:, None]
    glog[np.broadcast_to(smask[:, None, :], glog.shape)] = CRUSH

    fin = np.zeros((B, S), np.float32)
    fin[np.arange(B), 2 * L] = 1.0
    fin[np.arange(B), 2 * L - 1] = 1.0

    # forward stream: rounds t=0..63; backward stream (reversed t and s):
    # round j applies p at time 127-j, state 50-r.
    glogF = np.ascontiguousarray(glog[:, 0:TH, :])  # [B, 64, 51]
    glogB = np.ascontiguousarray(glog[:, TH:T, :][:, ::-1, ::-1])

    # Per-sample/direction prescale keeps the renorm-free f32 DP in range;
    # corrected exactly on the host.  The alpha row-sum grows per step by
    # the alpha-weighted 3-term branch sum ~ 2.5*mean(p~), not the full
    # row sum ~ (2L+1)*mean(p~), hence the L-correction; -0.22 centers
    # the residual Lyapunov drift (calibrated on N(0,1) logits, f32 has
    # +-88 nats of headroom against a +-25 observed spread).
    def prescale(g):
        m = g.max(axis=2, keepdims=True)
        rs = np.log(np.exp(g - m).sum(axis=2, keepdims=True)) + m
        rhat = rs.mean(axis=1, keepdims=True) + (
            np.log(2.5) - np.log(2.0 * L + 1.0) - 0.22
        )[:, None, None]
        rhat = rhat.astype(np.float32)
        return (g - rhat).astype(np.float32), rhat[:, 0, 0].astype(np.float64)

    glogF, rhatF = prescale(glogF)
    glogB, rhatB = prescale(glogB)

    skipf = skip.astype(np.float32)
    mF = np.zeros((B, S), np.float32)  # fwd skip mask at destination state s
    mF[:, 2:] = skipf[:, 2:]
    mBw = np.zeros((B, S), np.float32)  # bwd: mask at dest r is skip[52-r]
    mBw[:, 2:] = skipf[:, ::-1][:, :-2]

    a0F = np.zeros((B, S), np.float32)
    a0F[:, 0:2] = np.exp(glogF[:, 0, 0:2])
    a0B = np.exp(glogB[:, 0, :]) * fin[:, ::-1]  # E_127 = p~_127 * fin (rev)

    # state-major series [B, S, TH] -> [B, S*TH]
    glogFT = np.ascontiguousarray(np.transpose(glogF, (0, 2, 1))).reshape(B, S * TH)
    glogBT = np.ascontiguousarray(np.transpose(glogB, (0, 2, 1))).reshape(B, S * TH)

    return {
        "skip": skipf,
        "glogFT": glogFT,
        "glogBT": glogBT,
        "rhatF": rhatF,
        "rhatB": rhatB,
        "mF": mF,
        "mB": mBw,
        "a0F": a0F,
        "a0B": a0B,
    }


def _quantize_pred(pred):
    """Per-core pre-scaled bf16 upload: bf16(pred * 128*log2e), padded."""
    outs = []
    padded = np.zeros((BL * T, C2), np.float32)
    for m in range(NCORES):
        sl = pred[m * BL : (m + 1) * BL].reshape(BL * T, C)
        padded[:, :C] = sl * np.float32(A16)
        outs.append(padded.astype(ml_dtypes.bfloat16))
    return outs


def _core_in_map(p16s, hp, m):
    sl = slice(m * BL, (m + 1) * BL)
    return {
        "pred16": p16s[m],
        "glog": np.ascontiguousarray(
            np.concatenate([hp["glogFT"][sl], hp["glogBT"][sl]], 0)
        ),
        "m2": np.ascontiguousarray(np.concatenate([hp["mF"][sl], hp["mB"][sl]], 0)),
        "alpha0": np.ascontiguousarray(
            np.concatenate([hp["a0F"][sl], hp["a0B"][sl]], 0)
        ),
    }


def _combine(res_m, hp, L, m):
    """Junction + log bookkeeping for one core's outputs (float64 host math)."""
    sl = slice(m * BL, (m + 1) * BL)
    sume = np.asarray(res_m["sumexp"], np.float64)  # [T, BL]
    af = np.asarray(res_m["alpha_f"], np.float64)  # [BL2, S]
    # Pad-column corrections: exp(0)=1 on ACT rows, trick(0) on DVE rows.
    sume[:, :NA] -= 1.0
    sume[:, NA:] -= TRICK_PAD
    A = af[0:BL]  # alpha_63, fwd state coords  [BL, S]
    E = af[BL:BL2]  # D_64 in reversed coords     [BL, S]
    skip_r = hp["skip"][sl][:, ::-1].astype(np.float64)  # skip[50-r]

    # B_63 in reversed coords: B[r] = E[r] + E[r-1] + (E*skip_r)[r-2]
    GE = E * skip_r
    Brev = E.copy()
    Brev[:, 1:] += E[:, :-1]
    Brev[:, 2:] += GE[:, :-2]
    Bfwd = Brev[:, ::-1]  # back to fwd state coords

    lik = (A * Bfwd).sum(axis=1)
    dp_log = np.log(lik) + 64.0 * (hp["rhatF"][sl] + hp["rhatB"][sl])
    logZ = np.log(sume).sum(axis=0)  # [BL]
    Lm = L[sl]
    return -(dp_log - logZ) / Lm


def kernel(pred: np.ndarray, label: np.ndarray, label_length: np.ndarray) -> np.ndarray:
    global _LAST_RESULTS
    pred = np.ascontiguousarray(np.asarray(pred, dtype=np.float32))
    label = np.asarray(label)
    L = np.asarray(label_length).astype(np.int64)
    assert pred.shape == (B, T, C)

    hp = _host_prep(pred, label, L)
    p16s = _quantize_pred(pred)
    nc = _get_program()
    in_maps = [_core_in_map(p16s, hp, m) for m in range(NCORES)]
    out = run_bass_kernel_spmd(nc, in_maps, list(range(NCORES)), trace=_TRACE)
    _LAST_RESULTS = out
    res = out.results

    per_sample = [_combine(res[m], hp, L, m) for m in range(NCORES)]
    loss = np.concatenate(per_sample).mean()
    return np.float32(loss)

